# revision 1
# baseline (speedup 1.0000x reference)
"""ConditionalGNN (2-layer GCN + condition concat) on 8 trn2 NeuronCores.

Strategy (node-parallel with per-layer AllGather of the message table):
  - Math: with dinv = deg^-1/2 (deg includes self-loop),
      gcn(h)[d] = dinv[d] * sum_{e: dst=d} dinv[src_e] * (h @ W)[src_e] + b
    All biases are zero for this problem (asserted), which lets every dinv
    application be folded to a per-partition scalar multiply:
      g1 = dinv * (x @ W1u + condW1[batch])         (node-major, per-core shard)
      s1_T[c,d] = sum_e G1[e,c] * onehot(dstoff_e)[d]   (PSUM accumulation)
      h2raw_T = relu(s1_T)                           (channel-major)
      g2 = (1/deg) * (h2raw_T.T @ W2)                (node-major)
      s2_T likewise; logits[d] = dinv[d] * (relu(s2_T)[:,d] @ W_out)
  - Edges (+self-loops) are partitioned by dst core, grouped into dst-blocks
    of 128, padded to a uniform tile count so one SPMD program serves all
    cores. Messages are fetched with dma_gather (int16 idxs -> two table
    windows at row 32768). The one-hot selection matrices are built on DVE
    via tensor_scalar is_equal against an iota row.
  - bf16 data path, fp32 PSUM accumulation.

Self-contained: hardcodes all shapes from the problem spec.
"""

import numpy as np
import ml_dtypes

import concourse.bass as bass
from concourse import bacc
import concourse.mybir as mybir
import concourse.tile as tile
from concourse.bass_utils import run_bass_kernel_spmd

BF16 = ml_dtypes.bfloat16

N = 50000
NCORES = 8
NPC_REAL = N // NCORES          # 6250
NBLK = 49
NPC = NBLK * 128                # 6272 padded nodes per core
V = NCORES * NPC                # 50176 padded global nodes
CH = 128
FEAT = 768
KC = FEAT // 128                # 6 feature chunks
NG = 64                         # graphs
WINDOW = 32768                  # int16 index limit for dma_gather
GROUP_TILES = 8                 # gather tiles per dma_gather call (max)

dt = mybir.dt
Alu = mybir.AluOpType
Act = mybir.ActivationFunctionType


# ----------------------------------------------------------------- host prep

def _gid(n):
    c = n // NPC_REAL
    return c * NPC + (n - c * NPC_REAL)


def _wrap_idxs(idx):
    """[n] int -> wrapped int16 [128, n//16] (idx i at [i%16, i//16], x8 cores)."""
    n = idx.shape[0]
    arr = np.zeros((16, n // 16), np.int16)
    arr[np.arange(n) % 16, np.arange(n) // 16] = idx.astype(np.int16)
    return np.tile(arr, (8, 1))


def prep_inputs(x, edge_index, substring_embed, batch, W1, b1, W2, b2, W_out, b_out):
    """Integer graph preprocessing + per-core shard construction."""
    assert not np.any(np.asarray(b1)) and not np.any(np.asarray(b2)) and not np.any(
        np.asarray(b_out)
    ), "nonzero biases not supported by this kernel"

    x = np.asarray(x)
    edge_index = np.asarray(edge_index)
    batch = np.asarray(batch)

    src = np.concatenate([edge_index[0], np.arange(N, dtype=np.int64)])
    dst = np.concatenate([edge_index[1], np.arange(N, dtype=np.int64)])
    deg = np.bincount(dst, minlength=N).astype(np.float32)  # includes self-loop

    gsrc = _gid(src)
    gdst = _gid(dst)
    dst_core = dst // NPC_REAL
    dst_local = gdst - dst_core * NPC
    dst_block = dst_local >> 7
    dst_off = dst_local & 127

    # bucket edges: per (core, block), lo/hi window lists
    lo_mask = gsrc < WINDOW
    per = {}
    for c in range(NCORES):
        cm = dst_core == c
        for w, wm in (("lo", cm & lo_mask), ("hi", cm & ~lo_mask)):
            s = gsrc[wm]
            b = dst_block[wm]
            o = dst_off[wm]
            order = np.argsort(b, kind="stable")
            s, b, o = s[order], b[order], o[order]
            bounds = np.searchsorted(b, np.arange(NBLK + 1))
            per[c, w] = (s, o, bounds)

    def tiles_needed(c, w):
        s, o, bounds = per[c, w]
        cnt = np.diff(bounds)
        return np.maximum(1, -(-cnt // 128))

    T_LO = int(max(tiles_needed(c, "lo").max() for c in range(NCORES)))
    T_HI = int(max(tiles_needed(c, "hi").max() for c in range(NCORES)))
    NT_LO = NBLK * T_LO
    NT_HI = NBLK * T_HI

    def build_stream(c, w, T):
        s, o, bounds = per[c, w]
        nt = NBLK * T
        idx = np.zeros(nt * 128, np.int64)
        off = np.full((128, nt), 255.0, np.float32)  # 255 => zero one-hot column
        base = 0 if w == "lo" else WINDOW
        for b in range(NBLK):
            lo, hi = bounds[b], bounds[b + 1]
            n = hi - lo
            t0 = b * T * 128
            idx[t0 : t0 + n] = s[lo:hi] - base
            col = np.arange(n)
            off[col % 128, b * T + col // 128] = o[lo:hi]
        return idx, off

    def group_sizes(nt):
        ngroups = -(-nt // GROUP_TILES)
        q, r = divmod(nt, ngroups)
        return [q + (1 if i < r else 0) for i in range(ngroups)]

    g_lo = group_sizes(NT_LO)
    g_hi = group_sizes(NT_HI)

    per_core = []
    for c in range(NCORES):
        lo_idx, lo_off = build_stream(c, "lo", T_LO)
        hi_idx, hi_off = build_stream(c, "hi", T_HI)

        def wrap_stream(idx_arr, sizes):
            out, p = [], 0
            for sz in sizes:
                out.append(_wrap_idxs(idx_arr[p * 128 : (p + sz) * 128]))
                p += sz
            return np.concatenate(out, axis=1)

        # node shard data
        n0 = c * NPC_REAL
        xs = np.zeros((FEAT, NPC), BF16)
        xs[:, :NPC_REAL] = x[n0 : n0 + NPC_REAL].T.astype(BF16)
        bt = np.zeros((64, NPC), BF16)
        bshard = batch[n0 : n0 + NPC_REAL].astype(np.int64)
        bt[bshard, np.arange(NPC_REAL)] = 1.0
        degc = np.ones((128, NBLK), np.float32)
        dshard = deg[n0 : n0 + NPC_REAL]
        j = np.arange(NPC_REAL)
        degc[j % 128, j // 128] = dshard

        per_core.append(
            {
                "xT": xs,
                "BT": bt,
                "deg_col": degc,
                "idx_lo": wrap_stream(lo_idx, g_lo),
                "idx_hi": wrap_stream(hi_idx, g_hi),
                "dstoff_lo": lo_off,
                "dstoff_hi": hi_off,
            }
        )

    shared = {
        "SET": np.ascontiguousarray(np.asarray(substring_embed).T).astype(BF16),
        "W1u": np.asarray(W1)[:FEAT].astype(BF16),
        "W1l": np.asarray(W1)[FEAT:].astype(BF16),
        "W2": np.asarray(W2).astype(BF16),
        "Wout": np.asarray(W_out).astype(BF16),
        "iota": np.broadcast_to(np.arange(128), (128, 128)).astype(BF16),
    }
    in_maps = [{**pc, **shared} for pc in per_core]
    struct = {"T_LO": T_LO, "T_HI": T_HI, "g_lo": g_lo, "g_hi": g_hi}
    return in_maps, struct


# --------------------------------------------------------------- bass kernel

def accum_groups(sizes):
    out, t0 = [], 0
    for sz in sizes:
        out.append((t0, sz))
        t0 += sz
    return out


def build_nc(struct):
    T_LO, T_HI = struct["T_LO"], struct["T_HI"]
    g_lo, g_hi = struct["g_lo"], struct["g_hi"]
    NT_LO, NT_HI = NBLK * T_LO, NBLK * T_HI

    nc = bacc.Bacc("TRN2", target_bir_lowering=False, debug=False, num_devices=NCORES)

    xT_d = nc.dram_tensor("xT", [FEAT, NPC], dt.bfloat16, kind="ExternalInput")
    BT_d = nc.dram_tensor("BT", [64, NPC], dt.bfloat16, kind="ExternalInput")
    deg_d = nc.dram_tensor("deg_col", [128, NBLK], dt.float32, kind="ExternalInput")
    ilo_d = nc.dram_tensor("idx_lo", [128, NT_LO * 8], dt.int16, kind="ExternalInput")
    ihi_d = nc.dram_tensor("idx_hi", [128, NT_HI * 8], dt.int16, kind="ExternalInput")
    olo_d = nc.dram_tensor("dstoff_lo", [128, NT_LO], dt.float32, kind="ExternalInput")
    ohi_d = nc.dram_tensor("dstoff_hi", [128, NT_HI], dt.float32, kind="ExternalInput")
    SET_d = nc.dram_tensor("SET", [FEAT, 64], dt.bfloat16, kind="ExternalInput")
    W1u_d = nc.dram_tensor("W1u", [FEAT, CH], dt.bfloat16, kind="ExternalInput")
    W1l_d = nc.dram_tensor("W1l", [FEAT, CH], dt.bfloat16, kind="ExternalInput")
    W2_d = nc.dram_tensor("W2", [CH, CH], dt.bfloat16, kind="ExternalInput")
    Wout_d = nc.dram_tensor("Wout", [CH, 1], dt.bfloat16, kind="ExternalInput")
    iota_d = nc.dram_tensor("iota", [128, 128], dt.bfloat16, kind="ExternalInput")
    out_d = nc.dram_tensor("logits_col", [128, NBLK], dt.float32, kind="ExternalOutput")

    with tile.TileContext(nc) as tc:
        with (
            tc.tile_pool(name="const", bufs=1) as cp,
            tc.tile_pool(name="dram", bufs=1, space="DRAM") as dp,
            tc.tile_pool(name="psA", bufs=3, space="PSUM") as psA,
            tc.tile_pool(name="psB", bufs=2, space="PSUM") as psB,
            tc.tile_pool(name="ev", bufs=3) as ev,
        ):
            # ---- constants / small inputs
            iota_t = cp.tile([128, 128], dt.bfloat16)
            nc.sync.dma_start(iota_t[:], iota_d[:])
            olo_t = cp.tile([128, NT_LO], dt.float32)
            nc.sync.dma_start(olo_t[:], olo_d[:])
            ohi_t = cp.tile([128, NT_HI], dt.float32)
            nc.sync.dma_start(ohi_t[:], ohi_d[:])
            ilo_t = cp.tile([128, NT_LO * 8], dt.int16)
            nc.sync.dma_start(ilo_t[:], ilo_d[:])
            ihi_t = cp.tile([128, NT_HI * 8], dt.int16)
            nc.sync.dma_start(ihi_t[:], ihi_d[:])
            w1u = []
            w1l = []
            sets = []
            for k in range(KC):
                t1 = cp.tile([128, CH], dt.bfloat16, name=f"w1u_{k}")
                nc.sync.dma_start(t1[:], W1u_d[k * 128 : (k + 1) * 128, :])
                w1u.append(t1)
                t2 = cp.tile([128, CH], dt.bfloat16, name=f"w1l_{k}")
                nc.sync.dma_start(t2[:], W1l_d[k * 128 : (k + 1) * 128, :])
                w1l.append(t2)
                t3 = cp.tile([128, 64], dt.bfloat16, name=f"set_{k}")
                nc.sync.dma_start(t3[:], SET_d[k * 128 : (k + 1) * 128, :])
                sets.append(t3)
            W2_t = cp.tile([CH, CH], dt.bfloat16)
            nc.sync.dma_start(W2_t[:], W2_d[:])
            Wout_t = cp.tile([CH, 1], dt.bfloat16)
            nc.sync.dma_start(Wout_t[:], Wout_d[:])

            deg_t = cp.tile([128, NBLK], dt.float32)
            nc.sync.dma_start(deg_t[:], deg_d[:])
            dinv2_t = cp.tile([128, NBLK], dt.float32)
            nc.vector.reciprocal(dinv2_t[:], deg_t[:])
            dinv_t = cp.tile([128, NBLK], dt.float32)
            nc.scalar.activation(dinv_t[:], dinv2_t[:], Act.Sqrt)

            # condW1 = substring_embed @ W1l : [64, CH]
            cond_ps = psB.tile([64, CH], dt.float32, tag="mm", space="PSUM")
            for k in range(KC):
                nc.tensor.matmul(
                    cond_ps[:], lhsT=sets[k][:], rhs=w1l[k][:],
                    start=(k == 0), stop=(k == KC - 1),
                )
            cond_t = cp.tile([64, CH], dt.bfloat16)
            nc.vector.tensor_copy(cond_t[:], cond_ps[:])

            # DRAM scratch tables
            g1_local = dp.tile([NPC, CH], dt.bfloat16)
            g1_full = dp.tile([V, CH], dt.bfloat16, addr_space="Shared")
            g2_local = dp.tile([NPC, CH], dt.bfloat16)
            g2_full = dp.tile([V, CH], dt.bfloat16, addr_space="Shared")

            # ---- phase 1: g1 = dinv * (x @ W1u + condW1[batch])
            with tc.tile_pool(name="xt", bufs=1) as xp:
                xts = []
                for k in range(KC):
                    t = xp.tile([128, NPC], dt.bfloat16, name=f"xT_{k}")
                    nc.sync.dma_start(t[:], xT_d[k * 128 : (k + 1) * 128, :])
                    xts.append(t)
                BT_t = xp.tile([64, NPC], dt.bfloat16)
                nc.sync.dma_start(BT_t[:], BT_d[:])

                for t in range(NBLK):
                    sl = bass.ts(t, 128)
                    ps = psB.tile([128, CH], dt.float32, tag="mm", space="PSUM")
                    for k in range(KC):
                        nc.tensor.matmul(
                            ps[:], lhsT=xts[k][:, sl], rhs=w1u[k][:],
                            start=(k == 0), stop=False,
                        )
                    nc.tensor.matmul(
                        ps[:], lhsT=BT_t[:, sl], rhs=cond_t[:],
                        start=False, stop=True,
                    )
                    g1t = ev.tile([128, CH], dt.bfloat16, tag="gout")
                    nc.vector.tensor_scalar(
                        out=g1t[:], in0=ps[:],
                        scalar1=dinv_t[:, t : t + 1], scalar2=None, op0=Alu.mult,
                    )
                    nc.sync.dma_start(g1_local[sl, :], g1t[:])

            # ---- aggregation layers
            def agg_layer(gp, table_full, layer):
                """Writes g2_local (layer 1) or logits (layer 2).

                Gather groups are emitted just before the first block that
                consumes them so Pool's buffer-reuse stalls can't deadlock
                against consumption order.
                """
                streams = {
                    "lo": (g_lo, ilo_t, table_full[:], T_LO),
                    "hi": (g_hi, ihi_t, table_full[WINDOW:, :], T_HI),
                }
                gather_tiles = {"lo": [], "hi": []}
                pending = {"lo": list(enumerate(accum_groups(g_lo))),
                           "hi": list(enumerate(accum_groups(g_hi)))}

                def emit_gathers(b):
                    for sname, (sizes, itile, src_ap, T) in streams.items():
                        while pending[sname] and pending[sname][0][1][0] // T <= b:
                            gi, (t0, sz) = pending[sname].pop(0)
                            gt = gp.tile(
                                [128, GROUP_TILES, CH], dt.bfloat16,
                                tag=f"g{sname}", bufs=3,
                                name=f"gath_{layer}_{sname}_{gi}",
                            )
                            nc.gpsimd.dma_gather(
                                out_ap=gt[:, :sz, :],
                                in_ap=src_ap,
                                idxs_ap=itile[:, t0 * 8 : (t0 + sz) * 8],
                                num_idxs=sz * 128,
                                num_idxs_reg=sz * 128,
                                elem_size=CH,
                            )
                            gather_tiles[sname].append((gt, t0, sz))

                for b in range(NBLK):
                    emit_gathers(min(b + 1, NBLK - 1))  # prefetch one block ahead
                    ps = psA.tile([CH, 128], dt.float32, tag="agg", space="PSUM")
                    nmm = T_LO + T_HI
                    mi = 0
                    for sname, T, offt in (("lo", T_LO, olo_t), ("hi", T_HI, ohi_t)):
                        for j in range(T):
                            tau = b * T + j
                            oh = ev.tile([128, 128], dt.bfloat16, tag="oh", bufs=8)
                            nc.vector.tensor_scalar(
                                out=oh[:], in0=iota_t[:],
                                scalar1=offt[:, tau : tau + 1], scalar2=None,
                                op0=Alu.is_equal,
                            )
                            for gt, t0, sz in gather_tiles[sname]:
                                if t0 <= tau < t0 + sz:
                                    gsl = gt[:, tau - t0, :]
                                    break
                            nc.tensor.matmul(
                                ps[:], lhsT=gsl, rhs=oh[:],
                                start=(mi == 0), stop=(mi == nmm - 1),
                            )
                            mi += 1
                    hT = ev.tile([CH, 128], dt.bfloat16, tag="hT")
                    nc.scalar.activation(hT[:], ps[:], Act.Relu)
                    if layer == 1:
                        ps2 = psB.tile([128, CH], dt.float32, tag="mm", space="PSUM")
                        nc.tensor.matmul(ps2[:], lhsT=hT[:], rhs=W2_t[:], start=True, stop=True)
                        g2t = ev.tile([128, CH], dt.bfloat16, tag="gout")
                        nc.vector.tensor_scalar(
                            out=g2t[:], in0=ps2[:],
                            scalar1=dinv2_t[:, b : b + 1], scalar2=None, op0=Alu.mult,
                        )
                        nc.sync.dma_start(g2_local[bass.ts(b, 128), :], g2t[:])
                    else:
                        psl = psB.tile([128, 1], dt.float32, tag="lg", space="PSUM")
                        nc.tensor.matmul(psl[:], lhsT=hT[:], rhs=Wout_t[:], start=True, stop=True)
                        nc.vector.tensor_scalar(
                            out=logits_t[:, b : b + 1], in0=psl[:],
                            scalar1=dinv_t[:, b : b + 1], scalar2=None, op0=Alu.mult,
                        )

            logits_t = cp.tile([128, NBLK], dt.float32)

            with tc.tile_pool(name="gath", bufs=1) as gp:
                nc.gpsimd.collective_compute(
                    "AllGather", Alu.bypass,
                    replica_groups=[list(range(NCORES))],
                    ins=[g1_local.opt()], outs=[g1_full.opt()],
                )
                agg_layer(gp, g1_full, layer=1)
                nc.gpsimd.collective_compute(
                    "AllGather", Alu.bypass,
                    replica_groups=[list(range(NCORES))],
                    ins=[g2_local.opt()], outs=[g2_full.opt()],
                )
                agg_layer(gp, g2_full, layer=2)

            nc.gpsimd.dma_start(out_d[:], logits_t[:])

    nc.compile()
    return nc


# ------------------------------------------------------------------- driver

_CACHE = {}


def run_device(in_maps, struct):
    key = (struct["T_LO"], struct["T_HI"])
    if key not in _CACHE:
        _CACHE[key] = build_nc(struct)
    nc = _CACHE[key]
    res = run_bass_kernel_spmd(nc, in_maps, core_ids=list(range(NCORES)))
    return [r["logits_col"] for r in res.results]


def assemble(logit_cols):
    parts = []
    for c in range(NCORES):
        col = logit_cols[c]  # [128, NBLK]
        parts.append(col.T.reshape(-1)[:NPC_REAL])
    return np.concatenate(parts).astype(np.float32)


def kernel(**inputs) -> np.ndarray:
    in_maps, struct = prep_inputs(**inputs)
    return assemble(run_device(in_maps, struct))



# revision 3
# speedup vs baseline: 1.3941x; 1.3941x over previous
"""ConditionalGNN (2-layer GCN + condition concat) on 8 trn2 NeuronCores.

Strategy (node-parallel, chunked-AllGather pipeline):
  - Math: with dinv = deg^-1/2 (deg includes self-loop),
      gcn(h)[d] = dinv[d] * sum_{e: dst=d} dinv[src_e] * (h @ W)[src_e] + b
    All biases are zero for this problem (asserted), which lets every dinv
    application be folded to a per-partition scalar multiply:
      g1 = dinv * (x @ W1u + condW1[batch])         (node-major, per-core shard)
      s1_T[c,d] = sum_e G1[e,c] * onehot(dstoff_e)[d]   (PSUM accumulation)
      h2raw_T = relu(s1_T)                           (channel-major)
      g2 = (1/deg) * (h2raw_T.T @ W2)                (node-major)
      s2_T likewise; logits[d] = dinv[d] * (relu(s2_T)[:,d] @ W_out)
  - Each per-core node shard is split into half A (25 dst-blocks, 3200 rows)
    and half B (24 blocks, 3072 rows). The message table is published with
    TWO AllGathers per layer (one per half), so the second collective of each
    layer overlaps with gather/matmul consumption of the first half, and the
    layer-2 collectives overlap the tail of layer-1 aggregation. Chunk tables
    are 25600/24576 rows — inside the int16 dma_gather index range, which
    also replaces the old lo/hi window split.
  - Edges (+self-loops) are partitioned by dst core, grouped into dst-blocks
    of 128, and split into A/B streams by source half. Per (stream, block)
    they are padded to a uniform tile count so one SPMD program serves all
    cores. Messages are fetched with dma_gather (int16 idxs); the one-hot
    selection matrices are built on DVE via tensor_scalar is_equal against an
    iota row. Stream-A partial sums park in SBUF while stream B arrives.
  - bf16 data path, fp32 PSUM accumulation.

Self-contained: hardcodes all shapes from the problem spec.
"""

import numpy as np
import ml_dtypes

import concourse.bass as bass
from concourse import bacc
import concourse.mybir as mybir
import concourse.tile as tile
from concourse.bass_utils import run_bass_kernel_spmd

BF16 = ml_dtypes.bfloat16

N = 50000
NCORES = 8
NPC_REAL = N // NCORES          # 6250
NBLK = 49
NPC = NBLK * 128                # 6272 padded nodes per core
CH = 128
FEAT = 768
KC = FEAT // 128                # 6 feature chunks
NG = 64                         # graphs
NBLK_A = 25                     # dst/src blocks in half A
NBLK_B = NBLK - NBLK_A          # 24
HALF_A = NBLK_A * 128           # 3200 rows per core in half A
HALF_B = NBLK_B * 128           # 3072
VA = NCORES * HALF_A            # 25600 rows in chunk-A table (< 32768)
VB = NCORES * HALF_B            # 24576 rows in chunk-B table (< 32768)
GROUP_TILES = 8                 # gather tiles per dma_gather call (max)

dt = mybir.dt
Alu = mybir.AluOpType
Act = mybir.ActivationFunctionType


# ----------------------------------------------------------------- host prep

def _wrap_idxs(idx):
    """[n] int -> wrapped int16 [128, n//16] (idx i at [i%16, i//16], x8 cores)."""
    n = idx.shape[0]
    arr = np.zeros((16, n // 16), np.int16)
    arr[np.arange(n) % 16, np.arange(n) // 16] = idx.astype(np.int16)
    return np.tile(arr, (8, 1))


def prep_inputs(x, edge_index, substring_embed, batch, W1, b1, W2, b2, W_out, b_out):
    """Integer graph preprocessing + per-core shard construction."""
    assert not np.any(np.asarray(b1)) and not np.any(np.asarray(b2)) and not np.any(
        np.asarray(b_out)
    ), "nonzero biases not supported by this kernel"

    x = np.asarray(x)
    edge_index = np.asarray(edge_index)
    batch = np.asarray(batch)

    src = np.concatenate([edge_index[0], np.arange(N, dtype=np.int64)])
    dst = np.concatenate([edge_index[1], np.arange(N, dtype=np.int64)])
    deg = np.bincount(dst, minlength=N).astype(np.float32)  # includes self-loop

    src_core = src // NPC_REAL
    src_loc = src - src_core * NPC_REAL
    in_a = src_loc < HALF_A
    # row index inside the chunk-A / chunk-B gather tables
    idx_a_all = src_core * HALF_A + src_loc
    idx_b_all = src_core * HALF_B + (src_loc - HALF_A)

    dst_core = dst // NPC_REAL
    dst_loc = dst - dst_core * NPC_REAL
    dst_block = dst_loc >> 7
    dst_off = dst_loc & 127

    per = {}
    for c in range(NCORES):
        cm = dst_core == c
        for w, wm in (("a", cm & in_a), ("b", cm & ~in_a)):
            s = np.where(in_a, idx_a_all, idx_b_all)[wm]
            b = dst_block[wm]
            o = dst_off[wm]
            order = np.argsort(b, kind="stable")
            s, b, o = s[order], b[order], o[order]
            bounds = np.searchsorted(b, np.arange(NBLK + 1))
            per[c, w] = (s, o, bounds)

    def tiles_needed(c, w):
        s, o, bounds = per[c, w]
        cnt = np.diff(bounds)
        return np.maximum(1, -(-cnt // 128))

    T_A = int(max(tiles_needed(c, "a").max() for c in range(NCORES)))
    T_B = int(max(tiles_needed(c, "b").max() for c in range(NCORES)))
    NT_A = NBLK * T_A
    NT_B = NBLK * T_B

    def build_stream(c, w, T):
        s, o, bounds = per[c, w]
        nt = NBLK * T
        idx = np.zeros(nt * 128, np.int64)
        off = np.full((128, nt), 255.0, np.float32)  # 255 => zero one-hot column
        for b in range(NBLK):
            lo, hi = bounds[b], bounds[b + 1]
            n = hi - lo
            t0 = b * T * 128
            idx[t0 : t0 + n] = s[lo:hi]
            col = np.arange(n)
            off[col % 128, b * T + col // 128] = o[lo:hi]
        return idx, off

    def group_sizes(nt):
        ngroups = -(-nt // GROUP_TILES)
        q, r = divmod(nt, ngroups)
        return [q + (1 if i < r else 0) for i in range(ngroups)]

    g_a = group_sizes(NT_A)
    g_b = group_sizes(NT_B)

    per_core = []
    for c in range(NCORES):
        a_idx, a_off = build_stream(c, "a", T_A)
        b_idx, b_off = build_stream(c, "b", T_B)

        def wrap_stream(idx_arr, sizes):
            out, p = [], 0
            for sz in sizes:
                out.append(_wrap_idxs(idx_arr[p * 128 : (p + sz) * 128]))
                p += sz
            return np.concatenate(out, axis=1)

        # node shard data
        n0 = c * NPC_REAL
        xs = np.zeros((FEAT, NPC), BF16)
        xs[:, :NPC_REAL] = x[n0 : n0 + NPC_REAL].T.astype(BF16)
        bt = np.zeros((64, NPC), BF16)
        bshard = batch[n0 : n0 + NPC_REAL].astype(np.int64)
        bt[bshard, np.arange(NPC_REAL)] = 1.0
        degc = np.ones((128, NBLK), np.float32)
        dshard = deg[n0 : n0 + NPC_REAL]
        j = np.arange(NPC_REAL)
        degc[j % 128, j // 128] = dshard

        per_core.append(
            {
                "xT": xs,
                "BT": bt,
                "deg_col": degc,
                "idx_a": wrap_stream(a_idx, g_a),
                "idx_b": wrap_stream(b_idx, g_b),
                "dstoff_a": a_off,
                "dstoff_b": b_off,
            }
        )

    shared = {
        "SET": np.ascontiguousarray(np.asarray(substring_embed).T).astype(BF16),
        "W1u": np.asarray(W1)[:FEAT].astype(BF16),
        "W1l": np.asarray(W1)[FEAT:].astype(BF16),
        "W2": np.asarray(W2).astype(BF16),
        "Wout": np.asarray(W_out).astype(BF16),
        "iota": np.broadcast_to(np.arange(128), (128, 128)).astype(BF16),
    }
    in_maps = [{**pc, **shared} for pc in per_core]
    struct = {"T_LO": T_A, "T_HI": T_B, "g_lo": g_a, "g_hi": g_b}
    return in_maps, struct


# --------------------------------------------------------------- bass kernel

def accum_groups(sizes):
    out, t0 = [], 0
    for sz in sizes:
        out.append((t0, sz))
        t0 += sz
    return out


def build_nc(struct):
    T_A, T_B = struct["T_LO"], struct["T_HI"]
    g_a, g_b = struct["g_lo"], struct["g_hi"]
    NT_A, NT_B = NBLK * T_A, NBLK * T_B

    nc = bacc.Bacc("TRN2", target_bir_lowering=False, debug=False, num_devices=NCORES)

    xT_d = nc.dram_tensor("xT", [FEAT, NPC], dt.bfloat16, kind="ExternalInput")
    BT_d = nc.dram_tensor("BT", [64, NPC], dt.bfloat16, kind="ExternalInput")
    deg_d = nc.dram_tensor("deg_col", [128, NBLK], dt.float32, kind="ExternalInput")
    ia_d = nc.dram_tensor("idx_a", [128, NT_A * 8], dt.int16, kind="ExternalInput")
    ib_d = nc.dram_tensor("idx_b", [128, NT_B * 8], dt.int16, kind="ExternalInput")
    oa_d = nc.dram_tensor("dstoff_a", [128, NT_A], dt.float32, kind="ExternalInput")
    ob_d = nc.dram_tensor("dstoff_b", [128, NT_B], dt.float32, kind="ExternalInput")
    SET_d = nc.dram_tensor("SET", [FEAT, 64], dt.bfloat16, kind="ExternalInput")
    W1u_d = nc.dram_tensor("W1u", [FEAT, CH], dt.bfloat16, kind="ExternalInput")
    W1l_d = nc.dram_tensor("W1l", [FEAT, CH], dt.bfloat16, kind="ExternalInput")
    W2_d = nc.dram_tensor("W2", [CH, CH], dt.bfloat16, kind="ExternalInput")
    Wout_d = nc.dram_tensor("Wout", [CH, 1], dt.bfloat16, kind="ExternalInput")
    iota_d = nc.dram_tensor("iota", [128, 128], dt.bfloat16, kind="ExternalInput")
    out_d = nc.dram_tensor("logits_col", [128, NBLK], dt.float32, kind="ExternalOutput")

    with tile.TileContext(nc) as tc:
        with (
            tc.tile_pool(name="const", bufs=1) as cp,
            tc.tile_pool(name="dram", bufs=1, space="DRAM") as dp,
            tc.tile_pool(name="psA", bufs=3, space="PSUM") as psA,
            tc.tile_pool(name="psB", bufs=2, space="PSUM") as psB,
            tc.tile_pool(name="ev", bufs=3) as ev,
        ):
            # ---- constants / small inputs
            iota_t = cp.tile([128, 128], dt.bfloat16)
            nc.sync.dma_start(iota_t[:], iota_d[:])
            oa_t = cp.tile([128, NT_A], dt.float32)
            nc.sync.dma_start(oa_t[:], oa_d[:])
            ob_t = cp.tile([128, NT_B], dt.float32)
            nc.sync.dma_start(ob_t[:], ob_d[:])
            ia_t = cp.tile([128, NT_A * 8], dt.int16)
            nc.sync.dma_start(ia_t[:], ia_d[:])
            ib_t = cp.tile([128, NT_B * 8], dt.int16)
            nc.sync.dma_start(ib_t[:], ib_d[:])
            w1u = []
            w1l = []
            sets = []
            for k in range(KC):
                t1 = cp.tile([128, CH], dt.bfloat16, name=f"w1u_{k}")
                nc.sync.dma_start(t1[:], W1u_d[k * 128 : (k + 1) * 128, :])
                w1u.append(t1)
                t2 = cp.tile([128, CH], dt.bfloat16, name=f"w1l_{k}")
                nc.sync.dma_start(t2[:], W1l_d[k * 128 : (k + 1) * 128, :])
                w1l.append(t2)
                t3 = cp.tile([128, 64], dt.bfloat16, name=f"set_{k}")
                nc.sync.dma_start(t3[:], SET_d[k * 128 : (k + 1) * 128, :])
                sets.append(t3)
            W2_t = cp.tile([CH, CH], dt.bfloat16)
            nc.sync.dma_start(W2_t[:], W2_d[:])
            Wout_t = cp.tile([CH, 1], dt.bfloat16)
            nc.sync.dma_start(Wout_t[:], Wout_d[:])

            deg_t = cp.tile([128, NBLK], dt.float32)
            nc.sync.dma_start(deg_t[:], deg_d[:])
            dinv2_t = cp.tile([128, NBLK], dt.float32)
            nc.vector.reciprocal(dinv2_t[:], deg_t[:])
            dinv_t = cp.tile([128, NBLK], dt.float32)
            nc.scalar.activation(dinv_t[:], dinv2_t[:], Act.Sqrt)

            # condW1 = substring_embed @ W1l : [64, CH]
            cond_ps = psB.tile([64, CH], dt.float32, tag="mm", space="PSUM")
            for k in range(KC):
                nc.tensor.matmul(
                    cond_ps[:], lhsT=sets[k][:], rhs=w1l[k][:],
                    start=(k == 0), stop=(k == KC - 1),
                )
            cond_t = cp.tile([64, CH], dt.bfloat16)
            nc.vector.tensor_copy(cond_t[:], cond_ps[:])

            # DRAM scratch tables (A/B halves, per layer)
            g1_locA = dp.tile([HALF_A, CH], dt.bfloat16)
            g1_locB = dp.tile([HALF_B, CH], dt.bfloat16)
            g1_fulA = dp.tile([VA, CH], dt.bfloat16, addr_space="Shared")
            g1_fulB = dp.tile([VB, CH], dt.bfloat16, addr_space="Shared")
            g2_locA = dp.tile([HALF_A, CH], dt.bfloat16)
            g2_locB = dp.tile([HALF_B, CH], dt.bfloat16)
            g2_fulA = dp.tile([VA, CH], dt.bfloat16, addr_space="Shared")
            g2_fulB = dp.tile([VB, CH], dt.bfloat16, addr_space="Shared")

            def ag(loc, ful):
                nc.gpsimd.collective_compute(
                    "AllGather", Alu.bypass,
                    replica_groups=[list(range(NCORES))],
                    ins=[loc.opt()], outs=[ful.opt()],
                )

            def loc_slice(locA, locB, b):
                if b < NBLK_A:
                    return locA[bass.ts(b, 128), :]
                return locB[bass.ts(b - NBLK_A, 128), :]

            # partial (stream-A) aggregation parking buffer, reused per layer
            part_t = cp.tile([CH, NBLK * 128], dt.bfloat16)

            logits_t = cp.tile([128, NBLK], dt.float32)

            # ---- phase 1: g1 = dinv * (x @ W1u + condW1[batch])
            with tc.tile_pool(name="xt", bufs=1) as xp:
                xts = []
                for k in range(KC):
                    t = xp.tile([128, NPC], dt.bfloat16, name=f"xT_{k}")
                    nc.sync.dma_start(t[:], xT_d[k * 128 : (k + 1) * 128, :])
                    xts.append(t)
                BT_t = xp.tile([64, NPC], dt.bfloat16)
                nc.sync.dma_start(BT_t[:], BT_d[:])

                for t in range(NBLK):
                    sl = bass.ts(t, 128)
                    ps = psB.tile([128, CH], dt.float32, tag="mm", space="PSUM")
                    for k in range(KC):
                        nc.tensor.matmul(
                            ps[:], lhsT=xts[k][:, sl], rhs=w1u[k][:],
                            start=(k == 0), stop=False,
                        )
                    nc.tensor.matmul(
                        ps[:], lhsT=BT_t[:, sl], rhs=cond_t[:],
                        start=False, stop=True,
                    )
                    g1t = ev.tile([128, CH], dt.bfloat16, tag="gout")
                    nc.vector.tensor_scalar(
                        out=g1t[:], in0=ps[:],
                        scalar1=dinv_t[:, t : t + 1], scalar2=None, op0=Alu.mult,
                    )
                    nc.sync.dma_start(loc_slice(g1_locA, g1_locB, t), g1t[:])
                    if t == NBLK_A - 1:
                        ag(g1_locA, g1_fulA)  # overlaps phase-1 half B
                ag(g1_locB, g1_fulB)

            # ---- aggregation passes
            def pass_A(gp, table_ful, layer):
                """Stream-A gathers + matmuls; partial sums parked in part_t."""
                gather_tiles = []
                pending = list(enumerate(accum_groups(g_a)))

                def emit(b):
                    while pending and pending[0][1][0] // T_A <= b:
                        gi, (t0, sz) = pending.pop(0)
                        gt = gp.tile(
                            [128, GROUP_TILES, CH], dt.bfloat16,
                            tag="ga", bufs=3, name=f"gath_{layer}_a_{gi}",
                        )
                        nc.gpsimd.dma_gather(
                            out_ap=gt[:, :sz, :],
                            in_ap=table_ful[:],
                            idxs_ap=ia_t[:, t0 * 8 : (t0 + sz) * 8],
                            num_idxs=sz * 128,
                            num_idxs_reg=sz * 128,
                            elem_size=CH,
                        )
                        gather_tiles.append((gt, t0, sz))

                for b in range(NBLK):
                    emit(min(b + 1, NBLK - 1))
                    ps = psA.tile([CH, 128], dt.float32, tag="agg", space="PSUM")
                    for j in range(T_A):
                        tau = b * T_A + j
                        oh = ev.tile([128, 128], dt.bfloat16, tag="oh", bufs=8)
                        nc.vector.tensor_scalar(
                            out=oh[:], in0=iota_t[:],
                            scalar1=oa_t[:, tau : tau + 1], scalar2=None,
                            op0=Alu.is_equal,
                        )
                        for gt, t0, sz in gather_tiles:
                            if t0 <= tau < t0 + sz:
                                gsl = gt[:, tau - t0, :]
                                break
                        nc.tensor.matmul(
                            ps[:], lhsT=gsl, rhs=oh[:],
                            start=(j == 0), stop=(j == T_A - 1),
                        )
                    nc.vector.tensor_copy(part_t[:, bass.ts(b, 128)], ps[:])

            def pass_B(gp, table_ful, layer):
                """Stream-B gathers + matmuls; combine with parked partials,
                relu, then layer epilogue per block."""
                gather_tiles = []
                pending = list(enumerate(accum_groups(g_b)))

                def emit(b):
                    while pending and pending[0][1][0] // T_B <= b:
                        gi, (t0, sz) = pending.pop(0)
                        gt = gp.tile(
                            [128, GROUP_TILES, CH], dt.bfloat16,
                            tag="gb", bufs=3, name=f"gath_{layer}_b_{gi}",
                        )
                        nc.gpsimd.dma_gather(
                            out_ap=gt[:, :sz, :],
                            in_ap=table_ful[:],
                            idxs_ap=ib_t[:, t0 * 8 : (t0 + sz) * 8],
                            num_idxs=sz * 128,
                            num_idxs_reg=sz * 128,
                            elem_size=CH,
                        )
                        gather_tiles.append((gt, t0, sz))

                for b in range(NBLK):
                    emit(min(b + 1, NBLK - 1))
                    ps = psA.tile([CH, 128], dt.float32, tag="agg", space="PSUM")
                    for j in range(T_B):
                        tau = b * T_B + j
                        oh = ev.tile([128, 128], dt.bfloat16, tag="oh", bufs=8)
                        nc.vector.tensor_scalar(
                            out=oh[:], in0=iota_t[:],
                            scalar1=ob_t[:, tau : tau + 1], scalar2=None,
                            op0=Alu.is_equal,
                        )
                        for gt, t0, sz in gather_tiles:
                            if t0 <= tau < t0 + sz:
                                gsl = gt[:, tau - t0, :]
                                break
                        nc.tensor.matmul(
                            ps[:], lhsT=gsl, rhs=oh[:],
                            start=(j == 0), stop=(j == T_B - 1),
                        )
                    st = ev.tile([CH, 128], dt.float32, tag="st")
                    nc.vector.tensor_tensor(
                        out=st[:], in0=ps[:], in1=part_t[:, bass.ts(b, 128)],
                        op=Alu.add,
                    )
                    hT = ev.tile([CH, 128], dt.bfloat16, tag="hT")
                    nc.scalar.activation(hT[:], st[:], Act.Relu)
                    if layer == 1:
                        ps2 = psB.tile([128, CH], dt.float32, tag="mm", space="PSUM")
                        nc.tensor.matmul(ps2[:], lhsT=hT[:], rhs=W2_t[:], start=True, stop=True)
                        g2t = ev.tile([128, CH], dt.bfloat16, tag="gout")
                        nc.vector.tensor_scalar(
                            out=g2t[:], in0=ps2[:],
                            scalar1=dinv2_t[:, b : b + 1], scalar2=None, op0=Alu.mult,
                        )
                        nc.sync.dma_start(loc_slice(g2_locA, g2_locB, b), g2t[:])
                        if b == NBLK_A - 1:
                            ag(g2_locA, g2_fulA)  # overlaps rest of pass B
                    else:
                        psl = psB.tile([128, 1], dt.float32, tag="lg", space="PSUM")
                        nc.tensor.matmul(psl[:], lhsT=hT[:], rhs=Wout_t[:], start=True, stop=True)
                        nc.vector.tensor_scalar(
                            out=logits_t[:, b : b + 1], in0=psl[:],
                            scalar1=dinv_t[:, b : b + 1], scalar2=None, op0=Alu.mult,
                        )

            with tc.tile_pool(name="gath", bufs=1) as gp:
                pass_A(gp, g1_fulA, layer=1)
                pass_B(gp, g1_fulB, layer=1)
                ag(g2_locB, g2_fulB)
                pass_A(gp, g2_fulA, layer=2)
                pass_B(gp, g2_fulB, layer=2)

            nc.gpsimd.dma_start(out_d[:], logits_t[:])

    nc.compile()
    return nc


# ------------------------------------------------------------------- driver

_CACHE = {}


def run_device(in_maps, struct):
    key = (struct["T_LO"], struct["T_HI"])
    if key not in _CACHE:
        _CACHE[key] = build_nc(struct)
    nc = _CACHE[key]
    res = run_bass_kernel_spmd(nc, in_maps, core_ids=list(range(NCORES)))
    return [r["logits_col"] for r in res.results]


def assemble(logit_cols):
    parts = []
    for c in range(NCORES):
        col = logit_cols[c]  # [128, NBLK]
        parts.append(col.T.reshape(-1)[:NPC_REAL])
    return np.concatenate(parts).astype(np.float32)


def kernel(**inputs) -> np.ndarray:
    in_maps, struct = prep_inputs(**inputs)
    return assemble(run_device(in_maps, struct))
